# revision 34
# baseline (speedup 1.0000x reference)
"""A3TGCN (encoder/decoder TGCN + attention) Trainium2 kernel, 8-core data-parallel.

Restructured math (per chain c in {enc, dec}, per sample):
  SXT[t]  = (S @ X_t)^T                      (prologue dense matmul, M<=8 feats)
  pre_g   = Ub_g^T @ H + A_g^T @ SXT[t] + c_g  (g in {z, r, h}; all in
            transposed layout: [hid|gate partitions, node free])
  z' = sigmoid(-pre_z) = 1 - z;  r = sigmoid(pre_r)
  ht = tanh(Ub_h^T @ (H*r) + A_h^T @ SXT[t] + c_h)
  H  = H + z' * (ht - H)
  acc += (p_t * Wc)^T @ H                     (folded attention + final linear)
out = relu(acc + lin_b), shape (B, OUT, N)

Sharding: batch B=16 over 8 cores (2 samples/core), graph + params replicated.
"""

import numpy as np
import ml_dtypes

import concourse.bass as bass
import concourse.mybir as mybir
import concourse.tile as tile
from concourse import bacc
from concourse.bass_utils import run_bass_kernel_spmd

F32 = mybir.dt.float32
BF16 = mybir.dt.bfloat16
AF = mybir.ActivationFunctionType
BF = ml_dtypes.bfloat16

B, T, N, M = 16, 12, 2000, 8
MF, HID, OUT = 4, 256, 12
NCORES = 8
BL = B // NCORES          # samples per core
PN = 2048                 # padded node count
KBN = PN // 128           # 16 node k-blocks
G3 = 3 * HID              # 768 folded gate columns
# per-sample node chunks (free-dim tiles)
CHUNKS = [(0, 512), (512, 512), (1024, 512), (1536, 464)]


def _chunks_all():
    out = []
    for s in range(BL):
        for (o, w) in CHUNKS:
            out.append((s, o, w, s * N + o))
    return out


CHUNKS_ALL = _chunks_all()  # (sample, off_in_sample, width, global_off)


def build_nc():
    nc = bacc.Bacc("TRN2", target_bir_lowering=False, debug=False,
                   enable_asserts=False, num_devices=NCORES)

    xe_d = nc.declare_dram_parameter("xe", [KBN, 128, BL * 108], BF16, isOutput=False)
    xd_d = nc.declare_dram_parameter("xd", [KBN, 128, 128], BF16, isOutput=False)
    st_d = nc.declare_dram_parameter("st", [KBN, 128, N], BF16, isOutput=False)
    ube_d = nc.declare_dram_parameter("ub_enc", [2, 128, G3], BF16, isOutput=False)
    ubd_d = nc.declare_dram_parameter("ub_dec", [2, 128, G3], BF16, isOutput=False)
    ae_d = nc.declare_dram_parameter("a_enc", [128, G3], BF16, isOutput=False)
    ad_d = nc.declare_dram_parameter("a_dec", [128, G3], BF16, isOutput=False)
    owe_d = nc.declare_dram_parameter("ow_enc", [128, 2 * T * 32], BF16, isOutput=False)
    owd_d = nc.declare_dram_parameter("ow_dec", [128, 2 * T * 32], BF16, isOutput=False)
    sel_d = nc.declare_dram_parameter("sel", [128, OUT], F32, isOutput=False)
    linb_d = nc.declare_dram_parameter("linb", [OUT, 1], F32, isOutput=False)
    out_d = nc.declare_dram_parameter("out", [BL, OUT, N], F32, isOutput=True)

    with tile.TileContext(nc) as tc:
        _build(tc, nc, xe_d, xd_d, st_d, ube_d, ubd_d, ae_d, ad_d,
               owe_d, owd_d, sel_d, linb_d, out_d)
    nc.compile()
    return nc


def _build(tc, nc, xe_d, xd_d, st_d, ube_d, ubd_d, ae_d, ad_d,
           owe_d, owd_d, sel_d, linb_d, out_d):
    from contextlib import ExitStack
    es = ExitStack()
    with es:
        persist = es.enter_context(tc.tile_pool(name="persist", bufs=1))

        # ---- persistent tiles -------------------------------------------
        ub_sb = {}
        for ch, d in (("e", ube_d), ("d", ubd_d)):
            for kb in range(2):
                tl = persist.tile([128, G3], BF16, tag=f"ub_{ch}{kb}", name=f"ub_{ch}{kb}")
                nc.sync.dma_start(out=tl[:], in_=d.ap()[kb])
                ub_sb[ch, kb] = tl
        a_sb = {}
        for ch, d in (("e", ae_d), ("d", ad_d)):
            tl = persist.tile([128, G3], BF16, tag=f"a_{ch}", name=f"a_{ch}")
            nc.sync.dma_start(out=tl[:], in_=d.ap()[:])
            a_sb[ch] = tl
        ow_sb = {}
        for ch, d in (("e", owe_d), ("d", owd_d)):
            tl = persist.tile([128, 2 * T * 32], BF16, tag=f"ow_{ch}", name=f"ow_{ch}")
            nc.sync.dma_start(out=tl[:], in_=d.ap()[:])
            ow_sb[ch] = tl
        sel_sb = persist.tile([128, OUT], F32, tag="sel", name="sel")
        nc.sync.dma_start(out=sel_sb[:], in_=sel_d.ap()[:])
        linb_sb = persist.tile([OUT, 1], F32, tag="linb", name="linb")
        nc.sync.dma_start(out=linb_sb[:], in_=linb_d.ap()[:])

        # compact SXT (prologue matmul output layout):
        #   enc per sample: rows t*9+f (f==8 -> ones), 108 rows
        #   dec: rows 64*s + t*5+f (f==4 -> ones)
        sxt_e = [persist.tile([128, N], BF16, tag=f"sxt_e{s}", name=f"sxt_e{s}") for s in range(BL)]
        sxt_d = persist.tile([128, N], BF16, tag="sxt_d", name="sxt_d")

        # hidden state, transposed layout: [hid-within-kb partitions,
        # (kb, BL*N) free] -- one tile per chain so elementwise ops cover
        # both kb halves in a single instruction
        H = {}
        Hs = {}
        for ch in ("e", "d"):
            tl = persist.tile([128, 2 * BL * N], BF16, tag=f"H_{ch}", name=f"H_{ch}")
            nc.vector.memset(tl[:], 0.0)
            Hs[ch] = tl
            for kb in range(2):
                H[ch, kb] = tl[:, kb * BL * N:(kb + 1) * BL * N]
        # attention-weighted output accumulator, 4 col-group bands:
        # band j (partitions 32j..32j+31) holds chain/kb combo j's partial
        acc = persist.tile([128, BL * N], F32, tag="acc", name="acc")
        nc.vector.memset(acc[:], 0.0)

        # ---- prologue: SXT = (S @ X)^T ----------------------------------
        # kb-outer streaming: S^T / X kb-tiles are DMA-streamed while 6
        # PSUM tiles accumulate; two rounds cover the 12 (mb, chunk) combos.
        with tc.tile_pool(name="xin", bufs=3) as xin_pool, \
             tc.tile_pool(name="stin", bufs=3) as st_pool, \
             tc.tile_pool(name="sx_ps", bufs=6, space="PSUM") as sxps_pool:
            mbs = [("e", 0, 108, sxt_e[0], 108),
                   ("e", 108, 108, sxt_e[1], 108),
                   ("d", 0, 128, sxt_d, 128)]
            combos = [(xsel, mo, mw, dst, drows, co, cw)
                      for (xsel, mo, mw, dst, drows) in mbs
                      for (co, cw) in CHUNKS]
            for rnd in range(2):
                part = combos[rnd * 6:rnd * 6 + 6]
                ps_tiles = [sxps_pool.tile([128, 512], F32, tag="sx_ps",
                                           name=f"sx_ps{rnd}_{i}")
                            for i in range(len(part))]
                for kb in range(KBN):
                    xe_t = xin_pool.tile([128, BL * 108], BF16, tag="xe_s",
                                         name=f"xe_s{rnd}_{kb}")
                    nc.sync.dma_start(out=xe_t[:], in_=xe_d.ap()[kb])
                    xd_t = xin_pool.tile([128, 128], BF16, tag="xd_s",
                                         name=f"xd_s{rnd}_{kb}")
                    nc.sync.dma_start(out=xd_t[:], in_=xd_d.ap()[kb])
                    st_t = st_pool.tile([128, N], BF16, tag="st_s",
                                        name=f"st_s{rnd}_{kb}")
                    nc.sync.dma_start(out=st_t[:], in_=st_d.ap()[kb])
                    for i, (xsel, mo, mw, dst, drows, co, cw) in enumerate(part):
                        xt = xe_t if xsel == "e" else xd_t
                        nc.tensor.matmul(
                            ps_tiles[i][:mw, :cw],
                            xt[:, mo:mo + mw],
                            st_t[:, co:co + cw],
                            start=(kb == 0), stop=(kb == KBN - 1))
                for i, (xsel, mo, mw, dst, drows, co, cw) in enumerate(part):
                    nc.scalar.copy(dst[:drows, co:co + cw], ps_tiles[i][:drows, :cw])

        # ---- recurrence --------------------------------------------------
        with tc.tile_pool(name="gps", bufs=3, space="PSUM") as gps_pool, \
             tc.tile_pool(name="po", bufs=2, space="PSUM") as po_pool, \
             tc.tile_pool(name="work", bufs=3) as work:

            def h2(ch, go, cw):
                # (128, 2, cw) view of both kb halves of H at node window go
                return Hs[ch].rearrange("p (k n) -> p k n", k=2)[:, :, go:go + cw]

            def hr2(tl, cw):
                # (128, 2, cw) view of a (128, 1024) work tile's kb halves
                return tl.rearrange("p (k n) -> p k n", k=2)[:, :, :cw]

            def r2(tl, cw):
                return tl.rearrange("p (k n) -> p k n", k=2)[:, :, :cw]

            def sxr_load(ch, t):
                # replicate step-t SX rows into all four 32-aligned PE
                # row-group slots so small-K SX matmuls can pack into
                # concurrent row groups (different tile_position[0]).
                tl = work.tile([128, BL * N], BF16, tag=f"sxr_{ch}",
                               name=f"sxr_{ch}", bufs=3)
                nf = (M if ch == "e" else MF) + 1
                for s in range(BL):
                    for slot in range(4):
                        if ch == "e":
                            src = sxt_e[s][9 * t:9 * t + 9, :]
                        else:
                            src = sxt_d[64 * s + 5 * t:64 * s + 5 * t + 5, :]
                        nc.sync.dma_start(
                            out=tl[32 * slot:32 * slot + nf,
                                   s * N:(s + 1) * N], in_=src)
                return tl

            def h_mms(ch, ps, gbase, rhs_pair, cw, go):
                # the four K=128 hidden-state passes of two gate col blocks
                for gb in range(2):
                    gcol = gbase + 128 * gb
                    o = 512 * gb + go
                    nc.tensor.matmul(ps[:, o:o + cw],
                                     ub_sb[ch, 0][:, gcol:gcol + 128],
                                     rhs_pair[0], start=True, stop=False)
                    nc.tensor.matmul(ps[:, o:o + cw],
                                     ub_sb[ch, 1][:, gcol:gcol + 128],
                                     rhs_pair[1], start=False, stop=False)

            def sx_mm(ch, sxr, ps, gbase, gb, slot, s, co, cw, go):
                # small-K SX pass in PE row group `slot` (concurrent packing)
                nf = (M if ch == "e" else MF) + 1
                sl = 32 * slot
                o = 512 * gb + go
                nc.tensor.matmul(ps[:, o:o + cw],
                                 a_sb[ch][sl:sl + nf,
                                          gbase + 128 * gb:gbase + 128 * gb + 128],
                                 sxr[sl:sl + nf, s * N + co:s * N + co + cw],
                                 start=False, stop=True,
                                 tile_position=(sl, 0))

            def phase_A(ch, t, sxr):
                # r psums + sigmoid + hr
                hrs = []
                for (s, co, cw, go) in CHUNKS_ALL:
                    rp = gps_pool.tile([128, 1024], F32, tag="gate_ps", name="gate_ps")
                    hpair = (H[ch, 0][:, go:go + cw], H[ch, 1][:, go:go + cw])
                    h_mms(ch, rp, HID, hpair, cw, 0)
                    sx_mm(ch, sxr, rp, HID, 0, 0, s, co, cw, 0)
                    sx_mm(ch, sxr, rp, HID, 1, 1, s, co, cw, 0)
                    r_sb = work.tile([128, 1024], BF16, tag="r_sb", name="r_sb")
                    nc.scalar.activation(r_sb[:, :], rp[:, :], AF.Sigmoid)
                    hr = work.tile([128, 1024], BF16, tag=f"hr_{ch}", name=f"hr_{ch}", bufs=9)
                    nc.vector.tensor_mul(hr2(hr, cw), h2(ch, go, cw), r2(r_sb, cw))
                    hrs.append(hr)
                return hrs

            def phase_BC(ch, t, hrs, sxr):
                # ht/z psums + activations + GRU update, chunk-pipelined.
                # The four SX passes issue back-to-back into distinct PE row
                # groups -> concurrent execution.
                for i, (s, co, cw, go) in enumerate(CHUNKS_ALL):
                    hr = hrs[i]
                    hp = gps_pool.tile([128, 1024], F32, tag="gate_ps", name="gate_ps")
                    h_mms(ch, hp, 2 * HID, (hr[:, :cw], hr[:, 512:512 + cw]), cw, 0)
                    zp = gps_pool.tile([128, 1024], F32, tag="gate_ps", name="gate_ps")
                    hpair = (H[ch, 0][:, go:go + cw], H[ch, 1][:, go:go + cw])
                    h_mms(ch, zp, 0, hpair, cw, 0)
                    sx_mm(ch, sxr, hp, 2 * HID, 0, 0, s, co, cw, 0)
                    sx_mm(ch, sxr, hp, 2 * HID, 1, 1, s, co, cw, 0)
                    sx_mm(ch, sxr, zp, 0, 0, 2, s, co, cw, 0)
                    sx_mm(ch, sxr, zp, 0, 1, 3, s, co, cw, 0)
                    ht_sb = work.tile([128, 1024], BF16, tag=f"ht_{ch}", name=f"ht_{ch}", bufs=4)
                    nc.scalar.activation(ht_sb[:, :], hp[:, :], AF.Tanh)
                    zp_sb = work.tile([128, 1024], BF16, tag=f"zp_{ch}", name=f"zp_{ch}", bufs=4)
                    nc.scalar.activation(zp_sb[:, :], zp[:, :], AF.Sigmoid,
                                         scale=-1.0)
                    d_sb = work.tile([128, 1024], BF16, tag="d_sb", name="d_sb", bufs=3)
                    p_sb = work.tile([128, 1024], BF16, tag="p_sb", name="p_sb", bufs=2)
                    nc.vector.tensor_sub(hr2(d_sb, cw), hr2(ht_sb, cw), h2(ch, go, cw))
                    nc.vector.tensor_mul(hr2(p_sb, cw), hr2(zp_sb, cw), hr2(d_sb, cw))
                    nc.vector.tensor_add(h2(ch, go, cw), h2(ch, go, cw), hr2(p_sb, cw))

            def phase_D(t):
                # acc += (p_t W_ch)^T @ H_ch for the 4 (chain, kb) combos,
                # packed into 4 concurrent PE column groups (bands)
                for (s, co, cw, go) in CHUNKS_ALL:
                    po = po_pool.tile([128, 512], F32, tag="po", name="po")
                    for ci, ch in enumerate(("e", "d")):
                        for kb in range(2):
                            band = 32 * (2 * ci + kb)
                            wcol = 32 * (2 * t + kb)
                            nc.tensor.matmul(po[band:band + 32, :cw],
                                             ow_sb[ch][:, wcol:wcol + 32],
                                             H[ch, kb][:, go:go + cw],
                                             start=True, stop=True,
                                             tile_position=(0, band))
                    nc.vector.tensor_add(acc[:, go:go + cw], acc[:, go:go + cw],
                                         po[:, :cw])

            for t in range(T):
                sxr_e = sxr_load("e", t)
                sxr_d = sxr_load("d", t)
                hrs_e = phase_A("e", t, sxr_e)
                hrs_d = phase_A("d", t, sxr_d)
                phase_BC("e", t, hrs_e, sxr_e)
                phase_BC("d", t, hrs_d, sxr_d)
                phase_D(t)

        # ---- epilogue: out = relu(sel^T @ acc + lin_b) -------------------
        # sel sums the 4 accumulator bands back to the 12 output rows
        with tc.tile_pool(name="outp", bufs=2) as outp, \
             tc.tile_pool(name="eps", bufs=2, space="PSUM") as eps_pool:
            for (s, co, cw, go) in CHUNKS_ALL:
                bsp = eps_pool.tile([128, 512], F32, tag="bsp", name="bsp")
                nc.tensor.matmul(bsp[:OUT, :cw], sel_sb[:, :],
                                 acc[:, go:go + cw], start=True, stop=True)
                ot = outp.tile([OUT, 512], F32, tag="out_sb", name="out_sb")
                nc.scalar.activation(ot[:, :cw], bsp[:OUT, :cw], AF.Relu,
                                     bias=linb_sb[:, 0:1])
                nc.sync.dma_start(out=out_d.ap()[s, :, co:co + cw], in_=ot[:, :cw])


# ---------------------------------------------------------------------------
# host-side preparation
# ---------------------------------------------------------------------------

def _softmax(x):
    e = np.exp(x - x.max())
    return e / e.sum()


def _host_prep(inputs):
    f32 = np.float32
    src = np.concatenate([inputs["edge_index"][0].astype(np.int64),
                          np.arange(N, dtype=np.int64)])
    dst = np.concatenate([inputs["edge_index"][1].astype(np.int64),
                          np.arange(N, dtype=np.int64)])
    w = np.concatenate([inputs["edge_weights"].astype(f32),
                        np.ones(N, f32)])
    deg = np.zeros(N, f32)
    np.add.at(deg, dst, w)
    dinv = np.where(deg > 0, 1.0 / np.sqrt(deg), 0.0).astype(f32)
    norm = dinv[src] * w * dinv[dst]
    st = np.zeros((PN, N), f32)          # st[s, d] = S[d, s]
    np.add.at(st, (src, dst), norm)
    st[N, :] = 1.0                       # phantom src node -> ones row trick
    st_t = np.ascontiguousarray(st.reshape(KBN, 128, N).astype(BF))

    shared = {"st": st_t}
    for pfx, m_in, key in (("enc", M, "x_hist"), ("dec", MF, "x_forecast")):
        convW = inputs[f"{pfx}_convW"].astype(f32)
        convb = inputs[f"{pfx}_convb"].astype(f32)
        linW = inputs[f"{pfx}_linW"].astype(f32)
        linb = inputs[f"{pfx}_linb"].astype(f32)
        p = _softmax(inputs[f"{pfx}_att"].astype(f32))
        A = np.concatenate([convW[g] @ linW[g][:HID] for g in range(3)], axis=1)
        c = np.concatenate([convb[g] @ linW[g][:HID] + linb[g] for g in range(3)])
        Ub = np.concatenate([linW[g][HID:] for g in range(3)], axis=1)
        # A + bias row, replicated at the four 32-aligned PE row-group slots
        a_full = np.zeros((128, G3), f32)
        for sl in range(4):
            a_full[32 * sl:32 * sl + m_in] = A
            a_full[32 * sl + m_in] = c
        shared[f"a_{pfx}"] = np.ascontiguousarray(a_full.astype(BF))
        shared[f"ub_{pfx}"] = np.ascontiguousarray(
            Ub.reshape(2, 128, G3).astype(BF))
        Wc = inputs["lin_W"].astype(f32)[:HID] if pfx == "enc" \
            else inputs["lin_W"].astype(f32)[HID:]
        ow = np.zeros((128, 2 * T * 32), f32)
        for t in range(T):
            for kb in range(2):
                ow[:, 32 * (2 * t + kb):32 * (2 * t + kb) + OUT] = \
                    p[t] * Wc[128 * kb:128 * kb + 128]
        shared[f"ow_{pfx}"] = np.ascontiguousarray(ow.astype(BF))
    sel = np.zeros((128, OUT), f32)
    for j in range(4):
        for o in range(OUT):
            sel[32 * j + o, o] = 1.0
    shared["sel"] = sel
    shared["linb"] = np.ascontiguousarray(
        inputs["lin_b"].astype(f32).reshape(OUT, 1))

    # per-core X tensors
    xh = inputs["x_hist"].astype(f32)       # (B, T, N, M)
    xf = inputs["x_forecast"].astype(f32)   # (B, OUT, N, MF)
    in_maps = []
    for core in range(NCORES):
        sl = slice(core * BL, (core + 1) * BL)
        xe = np.zeros((PN, BL * 108), f32)
        for s in range(BL):
            b = core * BL + s
            for t in range(T):
                xe[:N, s * 108 + 9 * t:s * 108 + 9 * t + 8] = xh[b, t]
            xe[N, s * 108 + 9 * np.arange(T) + 8] = 1.0
        xd = np.zeros((PN, 128), f32)
        for s in range(BL):
            b = core * BL + s
            for t in range(T):
                xd[:N, 64 * s + 5 * t:64 * s + 5 * t + 4] = xf[b, t]
            xd[N, 64 * s + 5 * np.arange(T) + 4] = 1.0
        im = dict(shared)
        im["xe"] = np.ascontiguousarray(xe.reshape(KBN, 128, BL * 108).astype(BF))
        im["xd"] = np.ascontiguousarray(xd.reshape(KBN, 128, 128).astype(BF))
        in_maps.append(im)
    return in_maps


_NC_CACHE = None


def _get_nc():
    global _NC_CACHE
    if _NC_CACHE is None:
        _NC_CACHE = build_nc()
    return _NC_CACHE


def kernel(**inputs):
    inputs = {k: np.asarray(v) for k, v in inputs.items()}
    in_maps = _host_prep(inputs)
    nc = _get_nc()
    res = run_bass_kernel_spmd(nc, in_maps, list(range(NCORES)))
    outs = [res.results[i]["out"] for i in range(NCORES)]
    return np.concatenate(outs, axis=0).astype(np.float32)


if __name__ == "__main__":
    import reference as ref
    inputs = {k: np.asarray(v) for k, v in ref.setup_inputs().items()}
    got = kernel(**inputs)
    print("kernel out", got.shape, got.dtype)


# revision 35
# speedup vs baseline: 1.0274x; 1.0274x over previous
"""A3TGCN (encoder/decoder TGCN + attention) Trainium2 kernel, 8-core data-parallel.

Restructured math (per chain c in {enc, dec}, per sample):
  SXT[t]  = (S @ X_t)^T                      (prologue dense matmul, M<=8 feats)
  pre_g   = Ub_g^T @ H + A_g^T @ SXT[t] + c_g  (g in {z, r, h}; all in
            transposed layout: [hid|gate partitions, node free])
  z' = sigmoid(-pre_z) = 1 - z;  r = sigmoid(pre_r)
  ht = tanh(Ub_h^T @ (H*r) + A_h^T @ SXT[t] + c_h)
  H  = H + z' * (ht - H)
  acc += (p_t * Wc)^T @ H                     (folded attention + final linear)
out = relu(acc + lin_b), shape (B, OUT, N)

Sharding: batch B=16 over 8 cores (2 samples/core), graph + params replicated.
"""

import numpy as np
import ml_dtypes

import concourse.bass as bass
import concourse.mybir as mybir
import concourse.tile as tile
from concourse import bacc
from concourse.bass_utils import run_bass_kernel_spmd

F32 = mybir.dt.float32
BF16 = mybir.dt.bfloat16
AF = mybir.ActivationFunctionType
BF = ml_dtypes.bfloat16

B, T, N, M = 16, 12, 2000, 8
MF, HID, OUT = 4, 256, 12
NCORES = 8
BL = B // NCORES          # samples per core
PN = 2048                 # padded node count
KBN = PN // 128           # 16 node k-blocks
G3 = 3 * HID              # 768 folded gate columns
# per-sample node chunks (free-dim tiles)
CHUNKS = [(0, 512), (512, 512), (1024, 512), (1536, 464)]


def _chunks_all():
    out = []
    for s in range(BL):
        for (o, w) in CHUNKS:
            out.append((s, o, w, s * N + o))
    return out


CHUNKS_ALL = _chunks_all()  # (sample, off_in_sample, width, global_off)


def build_nc():
    nc = bacc.Bacc("TRN2", target_bir_lowering=False, debug=False,
                   enable_asserts=False, num_devices=NCORES)

    xe_d = nc.declare_dram_parameter("xe", [KBN, 128, BL * 108], BF16, isOutput=False)
    xd_d = nc.declare_dram_parameter("xd", [KBN, 128, 128], BF16, isOutput=False)
    st_d = nc.declare_dram_parameter("st", [KBN, 128, N], BF16, isOutput=False)
    ube_d = nc.declare_dram_parameter("ub_enc", [2, 128, G3], BF16, isOutput=False)
    ubd_d = nc.declare_dram_parameter("ub_dec", [2, 128, G3], BF16, isOutput=False)
    ae_d = nc.declare_dram_parameter("a_enc", [128, G3], BF16, isOutput=False)
    ad_d = nc.declare_dram_parameter("a_dec", [128, G3], BF16, isOutput=False)
    owe_d = nc.declare_dram_parameter("ow_enc", [128, 2 * T * 32], BF16, isOutput=False)
    owd_d = nc.declare_dram_parameter("ow_dec", [128, 2 * T * 32], BF16, isOutput=False)
    sel_d = nc.declare_dram_parameter("sel", [128, OUT], F32, isOutput=False)
    linb_d = nc.declare_dram_parameter("linb", [OUT, 1], F32, isOutput=False)
    out_d = nc.declare_dram_parameter("out", [BL, OUT, N], F32, isOutput=True)

    with tile.TileContext(nc) as tc:
        _build(tc, nc, xe_d, xd_d, st_d, ube_d, ubd_d, ae_d, ad_d,
               owe_d, owd_d, sel_d, linb_d, out_d)
    nc.compile()
    return nc


def _build(tc, nc, xe_d, xd_d, st_d, ube_d, ubd_d, ae_d, ad_d,
           owe_d, owd_d, sel_d, linb_d, out_d):
    from contextlib import ExitStack
    es = ExitStack()
    with es:
        persist = es.enter_context(tc.tile_pool(name="persist", bufs=1))

        # ---- persistent tiles -------------------------------------------
        ub_sb = {}
        for ch, d in (("e", ube_d), ("d", ubd_d)):
            for kb in range(2):
                tl = persist.tile([128, G3], BF16, tag=f"ub_{ch}{kb}", name=f"ub_{ch}{kb}")
                nc.sync.dma_start(out=tl[:], in_=d.ap()[kb])
                ub_sb[ch, kb] = tl
        a_sb = {}
        for ch, d in (("e", ae_d), ("d", ad_d)):
            tl = persist.tile([128, G3], BF16, tag=f"a_{ch}", name=f"a_{ch}")
            nc.sync.dma_start(out=tl[:], in_=d.ap()[:])
            a_sb[ch] = tl
        ow_sb = {}
        for ch, d in (("e", owe_d), ("d", owd_d)):
            tl = persist.tile([128, 2 * T * 32], BF16, tag=f"ow_{ch}", name=f"ow_{ch}")
            nc.sync.dma_start(out=tl[:], in_=d.ap()[:])
            ow_sb[ch] = tl
        sel_sb = persist.tile([128, OUT], F32, tag="sel", name="sel")
        nc.sync.dma_start(out=sel_sb[:], in_=sel_d.ap()[:])
        linb_sb = persist.tile([OUT, 1], F32, tag="linb", name="linb")
        nc.sync.dma_start(out=linb_sb[:], in_=linb_d.ap()[:])

        # compact SXT (prologue matmul output layout):
        #   enc per sample: rows t*9+f (f==8 -> ones), 108 rows
        #   dec: rows 64*s + t*5+f (f==4 -> ones)
        sxt_e = [persist.tile([128, N], BF16, tag=f"sxt_e{s}", name=f"sxt_e{s}") for s in range(BL)]
        sxt_d = persist.tile([128, N], BF16, tag="sxt_d", name="sxt_d")

        # hidden state, transposed layout: [hid-within-kb partitions,
        # (kb, BL*N) free] -- one tile per chain so elementwise ops cover
        # both kb halves in a single instruction
        H = {}
        Hs = {}
        for ch in ("e", "d"):
            tl = persist.tile([128, 2 * BL * N], BF16, tag=f"H_{ch}", name=f"H_{ch}")
            nc.vector.memset(tl[:], 0.0)
            Hs[ch] = tl
            for kb in range(2):
                H[ch, kb] = tl[:, kb * BL * N:(kb + 1) * BL * N]
        # attention-weighted output accumulator, 4 col-group bands:
        # band j (partitions 32j..32j+31) holds chain/kb combo j's partial
        acc = persist.tile([128, BL * N], F32, tag="acc", name="acc")
        nc.vector.memset(acc[:], 0.0)

        # ---- prologue: SXT = (S @ X)^T ----------------------------------
        # kb-outer streaming: S^T / X kb-tiles are DMA-streamed while 6
        # PSUM tiles accumulate; two rounds cover the 12 (mb, chunk) combos.
        with tc.tile_pool(name="xin", bufs=3) as xin_pool, \
             tc.tile_pool(name="stin", bufs=5) as st_pool, \
             tc.tile_pool(name="sx_ps", bufs=6, space="PSUM") as sxps_pool:
            mbs = [("e", 0, 108, sxt_e[0], 108),
                   ("e", 108, 108, sxt_e[1], 108),
                   ("d", 0, 128, sxt_d, 128)]
            combos = [(xsel, mo, mw, dst, drows, co, cw)
                      for (xsel, mo, mw, dst, drows) in mbs
                      for (co, cw) in CHUNKS]
            for rnd in range(2):
                part = combos[rnd * 6:rnd * 6 + 6]
                ps_tiles = [sxps_pool.tile([128, 512], F32, tag="sx_ps",
                                           name=f"sx_ps{rnd}_{i}")
                            for i in range(len(part))]
                for kb in range(KBN):
                    xe_t = xin_pool.tile([128, BL * 108], BF16, tag="xe_s",
                                         name=f"xe_s{rnd}_{kb}")
                    nc.sync.dma_start(out=xe_t[:], in_=xe_d.ap()[kb])
                    xd_t = xin_pool.tile([128, 128], BF16, tag="xd_s",
                                         name=f"xd_s{rnd}_{kb}")
                    nc.sync.dma_start(out=xd_t[:], in_=xd_d.ap()[kb])
                    st_t = st_pool.tile([128, N], BF16, tag="st_s",
                                        name=f"st_s{rnd}_{kb}")
                    nc.sync.dma_start(out=st_t[:], in_=st_d.ap()[kb])
                    for i, (xsel, mo, mw, dst, drows, co, cw) in enumerate(part):
                        xt = xe_t if xsel == "e" else xd_t
                        nc.tensor.matmul(
                            ps_tiles[i][:mw, :cw],
                            xt[:, mo:mo + mw],
                            st_t[:, co:co + cw],
                            start=(kb == 0), stop=(kb == KBN - 1))
                for i, (xsel, mo, mw, dst, drows, co, cw) in enumerate(part):
                    nc.scalar.copy(dst[:drows, co:co + cw], ps_tiles[i][:drows, :cw])

        # ---- recurrence --------------------------------------------------
        with tc.tile_pool(name="gps", bufs=4, space="PSUM") as gps_pool, \
             tc.tile_pool(name="work", bufs=3) as work:

            def h2(ch, go, cw):
                # (128, 2, cw) view of both kb halves of H at node window go
                return Hs[ch].rearrange("p (k n) -> p k n", k=2)[:, :, go:go + cw]

            def hr2(tl, cw):
                # (128, 2, cw) view of a (128, 1024) work tile's kb halves
                return tl.rearrange("p (k n) -> p k n", k=2)[:, :, :cw]

            def r2(tl, cw):
                return tl.rearrange("p (k n) -> p k n", k=2)[:, :, :cw]

            def sxr_load(ch, t):
                # replicate step-t SX rows into all four 32-aligned PE
                # row-group slots so small-K SX matmuls can pack into
                # concurrent row groups (different tile_position[0]).
                tl = work.tile([128, BL * N], BF16, tag=f"sxr_{ch}",
                               name=f"sxr_{ch}", bufs=3)
                nf = (M if ch == "e" else MF) + 1
                for s in range(BL):
                    for slot in range(4):
                        if ch == "e":
                            src = sxt_e[s][9 * t:9 * t + 9, :]
                        else:
                            src = sxt_d[64 * s + 5 * t:64 * s + 5 * t + 5, :]
                        nc.sync.dma_start(
                            out=tl[32 * slot:32 * slot + nf,
                                   s * N:(s + 1) * N], in_=src)
                return tl

            def h_mms(ch, ps, gbase, rhs_pair, cw, go):
                # the four K=128 hidden-state passes of two gate col blocks
                for gb in range(2):
                    gcol = gbase + 128 * gb
                    o = 512 * gb + go
                    nc.tensor.matmul(ps[:, o:o + cw],
                                     ub_sb[ch, 0][:, gcol:gcol + 128],
                                     rhs_pair[0], start=True, stop=False)
                    nc.tensor.matmul(ps[:, o:o + cw],
                                     ub_sb[ch, 1][:, gcol:gcol + 128],
                                     rhs_pair[1], start=False, stop=False)

            def sx_mm(ch, sxr, ps, gbase, gb, slot, s, co, cw, go):
                # small-K SX pass in PE row group `slot` (concurrent packing)
                nf = (M if ch == "e" else MF) + 1
                sl = 32 * slot
                o = 512 * gb + go
                nc.tensor.matmul(ps[:, o:o + cw],
                                 a_sb[ch][sl:sl + nf,
                                          gbase + 128 * gb:gbase + 128 * gb + 128],
                                 sxr[sl:sl + nf, s * N + co:s * N + co + cw],
                                 start=False, stop=True,
                                 tile_position=(sl, 0))

            def phase_A(ch, t, sxr):
                # r psums + sigmoid + hr
                hrs = []
                for (s, co, cw, go) in CHUNKS_ALL:
                    rp = gps_pool.tile([128, 1024], F32, tag="gate_ps", name="gate_ps")
                    hpair = (H[ch, 0][:, go:go + cw], H[ch, 1][:, go:go + cw])
                    h_mms(ch, rp, HID, hpair, cw, 0)
                    sx_mm(ch, sxr, rp, HID, 0, 0, s, co, cw, 0)
                    sx_mm(ch, sxr, rp, HID, 1, 1, s, co, cw, 0)
                    r_sb = work.tile([128, 1024], BF16, tag="r_sb", name="r_sb")
                    nc.scalar.activation(r_sb[:, :], rp[:, :], AF.Sigmoid)
                    hr = work.tile([128, 1024], BF16, tag=f"hr_{ch}", name=f"hr_{ch}", bufs=9)
                    nc.vector.tensor_mul(hr2(hr, cw), h2(ch, go, cw), r2(r_sb, cw))
                    hrs.append(hr)
                return hrs

            def phase_BC(ch, t, hrs, sxr):
                # ht/z psums + activations + GRU update, chunk-pipelined.
                # The four SX passes issue back-to-back into distinct PE row
                # groups -> concurrent execution.
                for i, (s, co, cw, go) in enumerate(CHUNKS_ALL):
                    hr = hrs[i]
                    hp = gps_pool.tile([128, 1024], F32, tag="gate_ps", name="gate_ps")
                    h_mms(ch, hp, 2 * HID, (hr[:, :cw], hr[:, 512:512 + cw]), cw, 0)
                    zp = gps_pool.tile([128, 1024], F32, tag="gate_ps", name="gate_ps")
                    hpair = (H[ch, 0][:, go:go + cw], H[ch, 1][:, go:go + cw])
                    h_mms(ch, zp, 0, hpair, cw, 0)
                    sx_mm(ch, sxr, hp, 2 * HID, 0, 0, s, co, cw, 0)
                    sx_mm(ch, sxr, hp, 2 * HID, 1, 1, s, co, cw, 0)
                    sx_mm(ch, sxr, zp, 0, 0, 2, s, co, cw, 0)
                    sx_mm(ch, sxr, zp, 0, 1, 3, s, co, cw, 0)
                    ht_sb = work.tile([128, 1024], BF16, tag=f"ht_{ch}", name=f"ht_{ch}", bufs=4)
                    nc.scalar.activation(ht_sb[:, :], hp[:, :], AF.Tanh)
                    zp_sb = work.tile([128, 1024], BF16, tag=f"zp_{ch}", name=f"zp_{ch}", bufs=4)
                    nc.scalar.activation(zp_sb[:, :], zp[:, :], AF.Sigmoid,
                                         scale=-1.0)
                    d_sb = work.tile([128, 1024], BF16, tag="d_sb", name="d_sb", bufs=3)
                    p_sb = work.tile([128, 1024], BF16, tag="p_sb", name="p_sb", bufs=2)
                    nc.vector.tensor_sub(hr2(d_sb, cw), hr2(ht_sb, cw), h2(ch, go, cw))
                    nc.vector.tensor_mul(hr2(p_sb, cw), hr2(zp_sb, cw), hr2(d_sb, cw))
                    nc.vector.tensor_add(h2(ch, go, cw), h2(ch, go, cw), hr2(p_sb, cw))

            def phase_D(t):
                # acc += (p_t W_ch)^T @ H_ch for the 4 (chain, kb) combos,
                # packed into 4 concurrent PE column groups (bands)
                for (s, co, cw, go) in CHUNKS_ALL:
                    po = gps_pool.tile([128, 1024], F32, tag="gate_ps", name="po")
                    for ci, ch in enumerate(("e", "d")):
                        for kb in range(2):
                            band = 32 * (2 * ci + kb)
                            wcol = 32 * (2 * t + kb)
                            nc.tensor.matmul(po[band:band + 32, :cw],
                                             ow_sb[ch][:, wcol:wcol + 32],
                                             H[ch, kb][:, go:go + cw],
                                             start=True, stop=True,
                                             tile_position=(0, band))
                    nc.vector.tensor_add(acc[:, go:go + cw], acc[:, go:go + cw],
                                         po[:, :cw])

            for t in range(T):
                sxr_e = sxr_load("e", t)
                sxr_d = sxr_load("d", t)
                hrs_e = phase_A("e", t, sxr_e)
                hrs_d = phase_A("d", t, sxr_d)
                phase_BC("e", t, hrs_e, sxr_e)
                phase_BC("d", t, hrs_d, sxr_d)
                phase_D(t)

        # ---- epilogue: out = relu(sel^T @ acc + lin_b) -------------------
        # sel sums the 4 accumulator bands back to the 12 output rows
        with tc.tile_pool(name="outp", bufs=2) as outp, \
             tc.tile_pool(name="eps", bufs=2, space="PSUM") as eps_pool:
            for (s, co, cw, go) in CHUNKS_ALL:
                bsp = eps_pool.tile([128, 512], F32, tag="bsp", name="bsp")
                nc.tensor.matmul(bsp[:OUT, :cw], sel_sb[:, :],
                                 acc[:, go:go + cw], start=True, stop=True)
                ot = outp.tile([OUT, 512], F32, tag="out_sb", name="out_sb")
                nc.scalar.activation(ot[:, :cw], bsp[:OUT, :cw], AF.Relu,
                                     bias=linb_sb[:, 0:1])
                nc.sync.dma_start(out=out_d.ap()[s, :, co:co + cw], in_=ot[:, :cw])


# ---------------------------------------------------------------------------
# host-side preparation
# ---------------------------------------------------------------------------

def _softmax(x):
    e = np.exp(x - x.max())
    return e / e.sum()


def _host_prep(inputs):
    f32 = np.float32
    src = np.concatenate([inputs["edge_index"][0].astype(np.int64),
                          np.arange(N, dtype=np.int64)])
    dst = np.concatenate([inputs["edge_index"][1].astype(np.int64),
                          np.arange(N, dtype=np.int64)])
    w = np.concatenate([inputs["edge_weights"].astype(f32),
                        np.ones(N, f32)])
    deg = np.zeros(N, f32)
    np.add.at(deg, dst, w)
    dinv = np.where(deg > 0, 1.0 / np.sqrt(deg), 0.0).astype(f32)
    norm = dinv[src] * w * dinv[dst]
    st = np.zeros((PN, N), f32)          # st[s, d] = S[d, s]
    np.add.at(st, (src, dst), norm)
    st[N, :] = 1.0                       # phantom src node -> ones row trick
    st_t = np.ascontiguousarray(st.reshape(KBN, 128, N).astype(BF))

    shared = {"st": st_t}
    for pfx, m_in, key in (("enc", M, "x_hist"), ("dec", MF, "x_forecast")):
        convW = inputs[f"{pfx}_convW"].astype(f32)
        convb = inputs[f"{pfx}_convb"].astype(f32)
        linW = inputs[f"{pfx}_linW"].astype(f32)
        linb = inputs[f"{pfx}_linb"].astype(f32)
        p = _softmax(inputs[f"{pfx}_att"].astype(f32))
        A = np.concatenate([convW[g] @ linW[g][:HID] for g in range(3)], axis=1)
        c = np.concatenate([convb[g] @ linW[g][:HID] + linb[g] for g in range(3)])
        Ub = np.concatenate([linW[g][HID:] for g in range(3)], axis=1)
        # A + bias row, replicated at the four 32-aligned PE row-group slots
        a_full = np.zeros((128, G3), f32)
        for sl in range(4):
            a_full[32 * sl:32 * sl + m_in] = A
            a_full[32 * sl + m_in] = c
        shared[f"a_{pfx}"] = np.ascontiguousarray(a_full.astype(BF))
        shared[f"ub_{pfx}"] = np.ascontiguousarray(
            Ub.reshape(2, 128, G3).astype(BF))
        Wc = inputs["lin_W"].astype(f32)[:HID] if pfx == "enc" \
            else inputs["lin_W"].astype(f32)[HID:]
        ow = np.zeros((128, 2 * T * 32), f32)
        for t in range(T):
            for kb in range(2):
                ow[:, 32 * (2 * t + kb):32 * (2 * t + kb) + OUT] = \
                    p[t] * Wc[128 * kb:128 * kb + 128]
        shared[f"ow_{pfx}"] = np.ascontiguousarray(ow.astype(BF))
    sel = np.zeros((128, OUT), f32)
    for j in range(4):
        for o in range(OUT):
            sel[32 * j + o, o] = 1.0
    shared["sel"] = sel
    shared["linb"] = np.ascontiguousarray(
        inputs["lin_b"].astype(f32).reshape(OUT, 1))

    # per-core X tensors
    xh = inputs["x_hist"].astype(f32)       # (B, T, N, M)
    xf = inputs["x_forecast"].astype(f32)   # (B, OUT, N, MF)
    in_maps = []
    for core in range(NCORES):
        sl = slice(core * BL, (core + 1) * BL)
        xe = np.zeros((PN, BL * 108), f32)
        for s in range(BL):
            b = core * BL + s
            for t in range(T):
                xe[:N, s * 108 + 9 * t:s * 108 + 9 * t + 8] = xh[b, t]
            xe[N, s * 108 + 9 * np.arange(T) + 8] = 1.0
        xd = np.zeros((PN, 128), f32)
        for s in range(BL):
            b = core * BL + s
            for t in range(T):
                xd[:N, 64 * s + 5 * t:64 * s + 5 * t + 4] = xf[b, t]
            xd[N, 64 * s + 5 * np.arange(T) + 4] = 1.0
        im = dict(shared)
        im["xe"] = np.ascontiguousarray(xe.reshape(KBN, 128, BL * 108).astype(BF))
        im["xd"] = np.ascontiguousarray(xd.reshape(KBN, 128, 128).astype(BF))
        in_maps.append(im)
    return in_maps


_NC_CACHE = None


def _get_nc():
    global _NC_CACHE
    if _NC_CACHE is None:
        _NC_CACHE = build_nc()
    return _NC_CACHE


def kernel(**inputs):
    inputs = {k: np.asarray(v) for k, v in inputs.items()}
    in_maps = _host_prep(inputs)
    nc = _get_nc()
    res = run_bass_kernel_spmd(nc, in_maps, list(range(NCORES)))
    outs = [res.results[i]["out"] for i in range(NCORES)]
    return np.concatenate(outs, axis=0).astype(np.float32)


if __name__ == "__main__":
    import reference as ref
    inputs = {k: np.asarray(v) for k, v in ref.setup_inputs().items()}
    got = kernel(**inputs)
    print("kernel out", got.shape, got.dtype)
